# revision 16
# baseline (speedup 1.0000x reference)
"""LoCon1d (position-specific conv1d) Trainium2 kernel.

out[b,o,s] = sum_{c,k} xpad[b,c,s+k] * w[o,c,s,k] + bias[o,s]
shapes: x (16,64,1024) f32, w (64,64,1024,3) f32, bias (64,1024) f32.

Sequence-parallel over 8 cores, 128 positions each, split into two
halves (A: j, B: 64+j). Per window t the PE stationary is the
block-diagonal x tile [128, 32] (rows 0:64 = half-A channels feeding
cols 0:16, rows 64:128 = half-B channels feeding cols 16:32). Moving
operand is the weight block for every (pair, tap) that consumes window
t, so one matmul covers up to 3 position-pairs x 64 out-channels and
taps accumulate in PSUM via a sliding window over 8-pair groups.

PSUM layout: 2 banks of [128, 512]; bank = 32 pairs as 4 col-tiled
group slots (tile_position col 32q) x 8 pair-slots x 64 channels. A
single [8,128] x [8,512] matmul per bank writes the bias into all 512
cols with start=True, so every later tap matmul is a pure accumulate.

Weights and x travel as float8e3 (e3 exp / m4 mantissa), bias/out as
f16: measured end-to-end rel err ~1.1e-2 vs the f32 reference.
"""

import numpy as np

import concourse.bass as bass
import concourse.mybir as mybir
import concourse.tile as tile
from concourse import bacc, bass_utils

N_CORES = 8
B, CIN, COUT, S, K = 16, 64, 64, 1024, 3
SC = S // N_CORES          # positions per core (128)
H = SC // 2                # half length (64)
NG = 8                     # pair groups per core (8 pairs each)
TW = H + K - 1             # windows per half (66)
GPC = 1                    # weight groups per DMA chunk
NCH = NG // GPC            # weight DMA chunks (8)
NWARM = 6                  # dummy PE warm-up matmuls (HAM un-throttle)

# per-group matmul column offsets: block i covers pair-slots lo..hi
_BLK = []
_cofs = 0
for _i in range(10):
    _lo, _hi = max(0, _i - 2), min(7, _i)
    _BLK.append((_lo, _hi, _cofs))
    _cofs += 64 * (_hi - _lo + 1)
GCOLS = _cofs              # 1536

W_DT = "f8e3"

_DT = {"f16": mybir.dt.float16, "f8e3": mybir.dt.float8e3}


def _np_dt(dt):
    if dt == "f8e3":
        import ml_dtypes
        return ml_dtypes.float8_e3m4
    return np.float16


def build_bass(w_dt=W_DT):
    wdt = _DT[w_dt]
    f16 = mybir.dt.float16
    f32 = mybir.dt.float32
    nc = bacc.Bacc("TRN2", target_bir_lowering=False, debug=False,
                   num_devices=N_CORES)
    xz = nc.dram_tensor("xz", [128, TW, 2 * B], wdt, kind="ExternalInput")
    wq = nc.dram_tensor("wq", [128, NG, GCOLS], wdt, kind="ExternalInput")
    cb = nc.dram_tensor("cb", [8, 128 + 1024], f16, kind="ExternalInput")
    out = nc.dram_tensor("out", [2, 128, 512], f16, kind="ExternalOutput")

    with tile.TileContext(nc) as tc:
        with (
            tc.tile_pool(name="xpool", bufs=1) as xpool,
            tc.tile_pool(name="wpool", bufs=1) as wpool,
            tc.tile_pool(name="cpool", bufs=1) as cpool,
            tc.tile_pool(name="opool", bufs=1) as opool,
            tc.tile_pool(name="psum", bufs=1, space="PSUM") as pspool,
        ):
            cb_sb = cpool.tile([8, 128 + 1024], f16, tag="cb")
            nc.sync.dma_start(out=cb_sb[:, :], in_=cb.ap())

            # block-diagonal stationary x: [p, t, b_ext], zeros baked on
            # host so every lhsT slice [128, 32] is contiguous (fast LDW)
            xr = xpool.tile([128, TW, 2 * B], wdt, tag="xr")
            nc.sync.dma_start(out=xr[0:64, :, :], in_=xz.ap()[0:64, :, :])
            nc.scalar.dma_start(out=xr[64:128, :, :],
                                in_=xz.ap()[64:128, :, :])

            w_sb = []
            for ch in range(NCH):
                wt = wpool.tile([128, GPC, GCOLS], wdt, tag=f"wt{ch}")
                eng = nc.sync if ch % 2 == 0 else nc.scalar
                if ch >= NCH // 2:
                    # split late chunks so their first-half matmuls overlap
                    # the second half's transfer + completion latency
                    hh = GCOLS // 2
                    eng.dma_start(out=wt[:, :, 0:hh],
                                  in_=wq.ap()[:, ch:ch + 1, 0:hh])
                    eng.dma_start(out=wt[:, :, hh:GCOLS],
                                  in_=wq.ap()[:, ch:ch + 1, hh:GCOLS])
                else:
                    eng.dma_start(out=wt[:, :, :],
                                  in_=wq.ap()[:, ch * GPC:(ch + 1) * GPC, :])
                w_sb.append(wt)

            ps = [pspool.tile([128, 512], f32, name=f"ps{b}", tag=f"ps{b}")
                  for b in range(2)]
            if NWARM:
                # Matmuls on a scratch tile gated only by a tiny DVE memset:
                # they issue at body start and un-throttle the PE clock (HAM)
                # while the real operands are still in flight. Results land in
                # a scratch PSUM bank and are never read.
                warm_sb = cpool.tile([1, 512], f16, tag="warm")
                nc.vector.memset(warm_sb[:, :], 0.0)
                psw = pspool.tile([128, 512], f32, tag="psw")
                for _ in range(NWARM):
                    nc.tensor.matmul(psw[:, :], lhsT=warm_sb[:, 0:128],
                                     rhs=warm_sb[:, :],
                                     start=True, stop=True)
            for bank in range(2):
                nc.tensor.matmul(ps[bank][:, :], lhsT=cb_sb[:, 0:128],
                                 rhs=cb_sb[:, 128 + 512 * bank:
                                           128 + 512 * (bank + 1)],
                                 start=True, stop=False)

            ob = [opool.tile([128, 512], f16, name=f"ob{b}", tag=f"ob{b}")
                  for b in range(2)]
            for bank in range(2):
                for qp in range(2):             # col-slot pair index
                    for i in range(10):
                        lo, hi, cofs = _BLK[i]
                        wd = 64 * (hi - lo + 1)
                        for qq in range(2):
                            q = 2 * qp + qq
                            g = 4 * bank + q
                            t = 8 * g + i
                            last = (qp == 1 and i == 9 and qq == 1)
                            nc.tensor.matmul(
                                ps[bank][32 * q + 0:32 * q + 32,
                                         64 * lo:64 * (hi + 1)],
                                lhsT=xr[:, t, :],
                                rhs=w_sb[g][:, 0, cofs:cofs + wd],
                                start=False, stop=last,
                                tile_position=(0, 32 * q),
                            )
                nc.vector.tensor_copy(out=ob[bank][:, :],
                                      in_=ps[bank][:, :])
                eng = nc.scalar if bank == 0 else nc.sync
                eng.dma_start(out=out.ap()[bank, :, :], in_=ob[bank][:, :])
    nc.compile()
    return nc


def prep_inputs(input, weight, bias, w_dt=W_DT):
    """Host-side shard + relayout. Returns list of per-core input dicts."""
    wnp = _np_dt(w_dt)
    xpad = np.pad(np.asarray(input, np.float32), ((0, 0), (0, 0), (1, 1)))
    w = np.asarray(weight, np.float32).transpose(1, 2, 3, 0)  # (c, s, k, o)
    bias = np.asarray(bias, np.float32)

    ones = np.zeros((8, 128), np.float16)
    for r in range(8):
        m0 = 32 * (r // 2) + 16 * (r % 2)
        ones[r, m0:m0 + 16] = 1.0

    in_maps = []
    for core in range(N_CORES):
        s0 = core * SC
        xz = np.zeros((128, TW, 2 * B), np.float32)
        xz[0:64, :, 0:B] = xpad[:, :, s0:s0 + TW].transpose(1, 2, 0)
        xz[64:128, :, B:2 * B] = \
            xpad[:, :, s0 + H:s0 + H + TW].transpose(1, 2, 0)

        wq = np.empty((128, NG, GCOLS), np.float32)
        for g in range(NG):
            for i in range(10):
                lo, hi, cofs = _BLK[i]
                for slot in range(lo, hi + 1):
                    j = 8 * g + slot
                    k = i - slot
                    c0 = cofs + (slot - lo) * 64
                    wq[0:64, g, c0:c0 + 64] = w[:, s0 + j, k, :]
                    wq[64:128, g, c0:c0 + 64] = w[:, s0 + H + j, k, :]

        cb = np.empty((8, 128 + 1024), np.float16)
        cb[:, 0:128] = ones
        for r in range(8):
            q, half = r // 2, r % 2
            for bank in range(2):
                sl = s0 + 32 * bank + 8 * q + 64 * half
                cb[r, 128 + 512 * bank:128 + 512 * (bank + 1)] = \
                    bias[:, sl:sl + 8].T.reshape(512)

        in_maps.append({
            "xz": np.ascontiguousarray(xz.astype(wnp)),
            "wq": np.ascontiguousarray(wq.astype(wnp)),
            "cb": cb,
        })
    return in_maps


def assemble_output(results):
    full = np.empty((B, COUT, S), np.float32)
    for core, r in enumerate(results):
        s0 = core * SC
        oc = np.asarray(r["out"], np.float32)     # (2, 128, 512)
        oc = oc.reshape(2, 4, 2, B, 8, COUT)      # bank q half b slot o
        oc = oc.transpose(3, 5, 2, 0, 1, 4)       # b o half bank q slot
        full[:, :, s0:s0 + SC] = oc.reshape(B, COUT, SC)
    return full


_CACHED = {}


def run(inputs, w_dt=W_DT, trace=False):
    if w_dt not in _CACHED:
        _CACHED[w_dt] = build_bass(w_dt)
    nc = _CACHED[w_dt]
    in_maps = prep_inputs(inputs["input"], inputs["weight"], inputs["bias"],
                          w_dt)
    res = bass_utils.run_bass_kernel_spmd(
        nc, in_maps, core_ids=list(range(N_CORES)), trace=trace)
    return assemble_output(res.results), res


def kernel(input, weight, bias):
    out, _ = run({"input": input, "weight": weight, "bias": bias},
                 trace=False)
    return out


# revision 18
# speedup vs baseline: 1.1954x; 1.1954x over previous
"""LoCon1d (position-specific conv1d) Trainium2 kernel.

out[b,o,s] = sum_{c,k} xpad[b,c,s+k] * w[o,c,s,k] + bias[o,s]
shapes: x (16,64,1024) f32, w (64,64,1024,3) f32, bias (64,1024) f32.

Sequence-parallel over 8 cores, 128 positions each, split into two
halves (A: j, B: 64+j). Per window t the PE stationary is the
block-diagonal x tile [128, 32] (rows 0:64 = half-A channels feeding
cols 0:16, rows 64:128 = half-B channels feeding cols 16:32). Moving
operand is the weight block for every (pair, tap) that consumes window
t, so one matmul covers up to 3 position-pairs x 64 out-channels and
taps accumulate in PSUM via a sliding window over 8-pair groups.

PSUM layout: 2 banks of [128, 512]; bank = 32 pairs as 4 col-tiled
group slots (tile_position col 32q) x 8 pair-slots x 64 channels. A
single [8,128] x [8,512] matmul per bank writes the bias into all 512
cols with start=True, so every later tap matmul is a pure accumulate.

Weights and x travel as float8e3 (e3 exp / m4 mantissa), bias/out as
f16: measured end-to-end rel err ~1.1e-2 vs the f32 reference.
"""

import numpy as np

import concourse.bass as bass
import concourse.mybir as mybir
import concourse.tile as tile
from concourse import bacc, bass_utils

N_CORES = 8
B, CIN, COUT, S, K = 16, 64, 64, 1024, 3
SC = S // N_CORES          # positions per core (128)
H = SC // 2                # half length (64)
NG = 8                     # pair groups per core (8 pairs each)
TW = H + K - 1             # windows per half (66)
GPC = 1                    # weight groups per DMA chunk
NCH = NG // GPC            # weight DMA chunks (8)
NWARM = 8                  # dummy PE warm-up matmuls (HAM un-throttle)

# per-group matmul column offsets: block i covers pair-slots lo..hi
_BLK = []
_cofs = 0
for _i in range(10):
    _lo, _hi = max(0, _i - 2), min(7, _i)
    _BLK.append((_lo, _hi, _cofs))
    _cofs += 64 * (_hi - _lo + 1)
GCOLS = _cofs              # 1536

W_DT = "f8e3"

_DT = {"f16": mybir.dt.float16, "f8e3": mybir.dt.float8e3}


def _np_dt(dt):
    if dt == "f8e3":
        import ml_dtypes
        return ml_dtypes.float8_e3m4
    return np.float16


def build_bass(w_dt=W_DT):
    wdt = _DT[w_dt]
    f16 = mybir.dt.float16
    f32 = mybir.dt.float32
    nc = bacc.Bacc("TRN2", target_bir_lowering=False, debug=False,
                   num_devices=N_CORES)
    xz = nc.dram_tensor("xz", [128, TW, 2 * B], wdt, kind="ExternalInput")
    wq = nc.dram_tensor("wq", [128, NG, GCOLS], wdt, kind="ExternalInput")
    cb = nc.dram_tensor("cb", [8, 128 + 1024], f16, kind="ExternalInput")
    out = nc.dram_tensor("out", [2, 128, 512], f16, kind="ExternalOutput")

    with tile.TileContext(nc) as tc:
        with (
            tc.tile_pool(name="xpool", bufs=1) as xpool,
            tc.tile_pool(name="wpool", bufs=1) as wpool,
            tc.tile_pool(name="cpool", bufs=1) as cpool,
            tc.tile_pool(name="opool", bufs=1) as opool,
            tc.tile_pool(name="psum", bufs=1, space="PSUM") as pspool,
        ):
            cb_sb = cpool.tile([8, 128 + 1024], f16, tag="cb")
            nc.sync.dma_start(out=cb_sb[:, :], in_=cb.ap())

            # block-diagonal stationary x: [p, t, b_ext], zeros baked on
            # host so every lhsT slice [128, 32] is contiguous (fast LDW)
            xr = xpool.tile([128, TW, 2 * B], wdt, tag="xr")
            nc.sync.dma_start(out=xr[0:64, :, :], in_=xz.ap()[0:64, :, :])
            nc.scalar.dma_start(out=xr[64:128, :, :],
                                in_=xz.ap()[64:128, :, :])

            w_sb = []
            for ch in range(NCH):
                wt = wpool.tile([128, GPC, GCOLS], wdt, tag=f"wt{ch}")
                eng = nc.sync if ch % 2 == 0 else nc.scalar
                if ch >= 6:
                    # split the last chunk per queue at the i-block boundary
                    # (col 768 = end of block i=4) so its first-half matmuls
                    # overlap the second half's transfer + completion latency
                    hh = GCOLS // 2
                    eng.dma_start(out=wt[:, :, 0:hh],
                                  in_=wq.ap()[:, ch:ch + 1, 0:hh])
                    eng.dma_start(out=wt[:, :, hh:GCOLS],
                                  in_=wq.ap()[:, ch:ch + 1, hh:GCOLS])
                else:
                    eng.dma_start(out=wt[:, :, :],
                                  in_=wq.ap()[:, ch * GPC:(ch + 1) * GPC, :])
                w_sb.append(wt)

            ps = [pspool.tile([128, 512], f32, name=f"ps{b}", tag=f"ps{b}")
                  for b in range(2)]
            if NWARM:
                # Matmuls on a scratch tile gated only by a tiny DVE memset:
                # they issue at body start and un-throttle the PE clock (HAM)
                # while the real operands are still in flight. Results land in
                # a scratch PSUM bank and are never read.
                warm_sb = cpool.tile([1, 512], f16, tag="warm")
                nc.vector.memset(warm_sb[:, :], 0.0)
                psw = pspool.tile([128, 512], f32, tag="psw")
                for _ in range(NWARM):
                    nc.tensor.matmul(psw[:, :], lhsT=warm_sb[:, 0:128],
                                     rhs=warm_sb[:, :],
                                     start=True, stop=True)
            for bank in range(2):
                nc.tensor.matmul(ps[bank][:, :], lhsT=cb_sb[:, 0:128],
                                 rhs=cb_sb[:, 128 + 512 * bank:
                                           128 + 512 * (bank + 1)],
                                 start=True, stop=False)

            ob = [opool.tile([128, 512], f16, name=f"ob{b}", tag=f"ob{b}")
                  for b in range(2)]
            for bank in range(2):
                for qp in range(2):             # col-slot pair index
                    for i in range(10):
                        lo, hi, cofs = _BLK[i]
                        wd = 64 * (hi - lo + 1)
                        for qq in range(2):
                            q = 2 * qp + qq
                            g = 4 * bank + q
                            t = 8 * g + i
                            last = (qp == 1 and i == 9 and qq == 1)
                            nc.tensor.matmul(
                                ps[bank][32 * q + 0:32 * q + 32,
                                         64 * lo:64 * (hi + 1)],
                                lhsT=xr[:, t, :],
                                rhs=w_sb[g][:, 0, cofs:cofs + wd],
                                start=False, stop=last,
                                tile_position=(0, 32 * q),
                            )
                if bank == 0:
                    nc.vector.tensor_copy(out=ob[0][:, :], in_=ps[0][:, :])
                    nc.scalar.dma_start(out=out.ap()[0, :, :],
                                        in_=ob[0][:, :])
                else:
                    # split the final evacuation so its two out-DMAs issue on
                    # different engines concurrently
                    nc.vector.tensor_copy(out=ob[1][:, 0:256],
                                          in_=ps[1][:, 0:256])
                    nc.sync.dma_start(out=out.ap()[1, :, 0:256],
                                      in_=ob[1][:, 0:256])
                    nc.vector.tensor_copy(out=ob[1][:, 256:512],
                                          in_=ps[1][:, 256:512])
                    nc.scalar.dma_start(out=out.ap()[1, :, 256:512],
                                        in_=ob[1][:, 256:512])
    nc.compile()
    return nc


def prep_inputs(input, weight, bias, w_dt=W_DT):
    """Host-side shard + relayout. Returns list of per-core input dicts."""
    wnp = _np_dt(w_dt)
    xpad = np.pad(np.asarray(input, np.float32), ((0, 0), (0, 0), (1, 1)))
    w = np.asarray(weight, np.float32).transpose(1, 2, 3, 0)  # (c, s, k, o)
    bias = np.asarray(bias, np.float32)

    ones = np.zeros((8, 128), np.float16)
    for r in range(8):
        m0 = 32 * (r // 2) + 16 * (r % 2)
        ones[r, m0:m0 + 16] = 1.0

    in_maps = []
    for core in range(N_CORES):
        s0 = core * SC
        xz = np.zeros((128, TW, 2 * B), np.float32)
        xz[0:64, :, 0:B] = xpad[:, :, s0:s0 + TW].transpose(1, 2, 0)
        xz[64:128, :, B:2 * B] = \
            xpad[:, :, s0 + H:s0 + H + TW].transpose(1, 2, 0)

        wq = np.empty((128, NG, GCOLS), np.float32)
        for g in range(NG):
            for i in range(10):
                lo, hi, cofs = _BLK[i]
                for slot in range(lo, hi + 1):
                    j = 8 * g + slot
                    k = i - slot
                    c0 = cofs + (slot - lo) * 64
                    wq[0:64, g, c0:c0 + 64] = w[:, s0 + j, k, :]
                    wq[64:128, g, c0:c0 + 64] = w[:, s0 + H + j, k, :]

        cb = np.empty((8, 128 + 1024), np.float16)
        cb[:, 0:128] = ones
        for r in range(8):
            q, half = r // 2, r % 2
            for bank in range(2):
                sl = s0 + 32 * bank + 8 * q + 64 * half
                cb[r, 128 + 512 * bank:128 + 512 * (bank + 1)] = \
                    bias[:, sl:sl + 8].T.reshape(512)

        in_maps.append({
            "xz": np.ascontiguousarray(xz.astype(wnp)),
            "wq": np.ascontiguousarray(wq.astype(wnp)),
            "cb": cb,
        })
    return in_maps


def assemble_output(results):
    full = np.empty((B, COUT, S), np.float32)
    for core, r in enumerate(results):
        s0 = core * SC
        oc = np.asarray(r["out"], np.float32)     # (2, 128, 512)
        oc = oc.reshape(2, 4, 2, B, 8, COUT)      # bank q half b slot o
        oc = oc.transpose(3, 5, 2, 0, 1, 4)       # b o half bank q slot
        full[:, :, s0:s0 + SC] = oc.reshape(B, COUT, SC)
    return full


_CACHED = {}


def run(inputs, w_dt=W_DT, trace=False):
    if w_dt not in _CACHED:
        _CACHED[w_dt] = build_bass(w_dt)
    nc = _CACHED[w_dt]
    in_maps = prep_inputs(inputs["input"], inputs["weight"], inputs["bias"],
                          w_dt)
    res = bass_utils.run_bass_kernel_spmd(
        nc, in_maps, core_ids=list(range(N_CORES)), trace=trace)
    return assemble_output(res.results), res


def kernel(input, weight, bias):
    out, _ = run({"input": input, "weight": weight, "bias": bias},
                 trace=False)
    return out


# revision 20
# speedup vs baseline: 1.2375x; 1.0353x over previous
"""LoCon1d (position-specific conv1d) Trainium2 kernel.

out[b,o,s] = sum_{c,k} xpad[b,c,s+k] * w[o,c,s,k] + bias[o,s]
shapes: x (16,64,1024) f32, w (64,64,1024,3) f32, bias (64,1024) f32.

Sequence-parallel over 8 cores, 128 positions each, split into two
halves (A: j, B: 64+j). Per window t the PE stationary is the
block-diagonal x tile [128, 32] (rows 0:64 = half-A channels feeding
cols 0:16, rows 64:128 = half-B channels feeding cols 16:32). Moving
operand is the weight block for every (pair, tap) that consumes window
t, so one matmul covers up to 3 position-pairs x 64 out-channels and
taps accumulate in PSUM via a sliding window over 8-pair groups.

PSUM layout: 2 banks of [128, 512]; bank = 32 pairs as 4 col-tiled
group slots (tile_position col 32q) x 8 pair-slots x 64 channels. A
single [8,128] x [8,512] matmul per bank writes the bias into all 512
cols with start=True, so every later tap matmul is a pure accumulate.

Weights and x travel as float8e3 (e3 exp / m4 mantissa), bias/out as
f16: measured end-to-end rel err ~1.1e-2 vs the f32 reference.
"""

import numpy as np

import concourse.bass as bass
import concourse.mybir as mybir
import concourse.tile as tile
from concourse import bacc, bass_utils

N_CORES = 8
B, CIN, COUT, S, K = 16, 64, 64, 1024, 3
SC = S // N_CORES          # positions per core (128)
H = SC // 2                # half length (64)
NG = 8                     # pair groups per core (8 pairs each)
TW = H + K - 1             # windows per half (66)
GPC = 1                    # weight groups per DMA chunk
NCH = NG // GPC            # weight DMA chunks (8)
NWARM = 8                  # dummy PE warm-up matmuls (HAM un-throttle)

# per-group matmul column offsets: block i covers pair-slots lo..hi
_BLK = []
_cofs = 0
for _i in range(10):
    _lo, _hi = max(0, _i - 2), min(7, _i)
    _BLK.append((_lo, _hi, _cofs))
    _cofs += 64 * (_hi - _lo + 1)
GCOLS = _cofs              # 1536

W_DT = "f8e3"

_DT = {"f16": mybir.dt.float16, "f8e3": mybir.dt.float8e3}


def _np_dt(dt):
    if dt == "f8e3":
        import ml_dtypes
        return ml_dtypes.float8_e3m4
    return np.float16


def build_bass(w_dt=W_DT):
    wdt = _DT[w_dt]
    f16 = mybir.dt.float16
    f32 = mybir.dt.float32
    nc = bacc.Bacc("TRN2", target_bir_lowering=False, debug=False,
                   num_devices=N_CORES)
    xz = nc.dram_tensor("xz", [128, TW, 2 * B], wdt, kind="ExternalInput")
    wq = nc.dram_tensor("wq", [128, NG, GCOLS], wdt, kind="ExternalInput")
    cb = nc.dram_tensor("cb", [8, 128 + 1024], f16, kind="ExternalInput")
    out = nc.dram_tensor("out", [2, 128, 512], f16, kind="ExternalOutput")

    with tile.TileContext(nc) as tc:
        with (
            tc.tile_pool(name="xpool", bufs=1) as xpool,
            tc.tile_pool(name="wpool", bufs=1) as wpool,
            tc.tile_pool(name="cpool", bufs=1) as cpool,
            tc.tile_pool(name="opool", bufs=1) as opool,
            tc.tile_pool(name="psum", bufs=1, space="PSUM") as pspool,
        ):
            cb_sb = cpool.tile([8, 128 + 1024], f16, tag="cb")
            nc.sync.dma_start(out=cb_sb[:, :], in_=cb.ap())

            # block-diagonal stationary x: [p, t, b_ext], zeros baked on
            # host so every lhsT slice [128, 32] is contiguous (fast LDW)
            xr = xpool.tile([128, TW, 2 * B], wdt, tag="xr")
            nc.sync.dma_start(out=xr[0:64, :, :], in_=xz.ap()[0:64, :, :])
            nc.scalar.dma_start(out=xr[64:128, :, :],
                                in_=xz.ap()[64:128, :, :])

            w_sb = []
            for ch in range(NCH):
                wt = wpool.tile([128, GPC, GCOLS], wdt, tag=f"wt{ch}")
                eng = nc.sync if ch % 2 == 0 else nc.scalar
                if ch >= 6:
                    # split the last chunk per queue at the i-block boundary
                    # (col 768 = end of block i=4) so its first-half matmuls
                    # overlap the second half's transfer + completion latency
                    hh = GCOLS // 2
                    eng.dma_start(out=wt[:, :, 0:hh],
                                  in_=wq.ap()[:, ch:ch + 1, 0:hh])
                    eng.dma_start(out=wt[:, :, hh:GCOLS],
                                  in_=wq.ap()[:, ch:ch + 1, hh:GCOLS])
                else:
                    eng.dma_start(out=wt[:, :, :],
                                  in_=wq.ap()[:, ch * GPC:(ch + 1) * GPC, :])
                w_sb.append(wt)

            ps = [pspool.tile([128, 512], f32, name=f"ps{b}", tag=f"ps{b}")
                  for b in range(2)]
            if NWARM:
                # Matmuls on a scratch tile gated only by a tiny DVE memset:
                # they issue at body start and un-throttle the PE clock (HAM)
                # while the real operands are still in flight. Results land in
                # a scratch PSUM bank and are never read.
                warm_sb = cpool.tile([1, 512], f16, tag="warm")
                nc.vector.memset(warm_sb[:, :], 0.0)
                psw = pspool.tile([128, 512], f32, tag="psw")
                for _ in range(NWARM):
                    nc.tensor.matmul(psw[:, :], lhsT=warm_sb[:, 0:128],
                                     rhs=warm_sb[:, :],
                                     start=True, stop=True)
            for bank in range(2):
                nc.tensor.matmul(ps[bank][:, :], lhsT=cb_sb[:, 0:128],
                                 rhs=cb_sb[:, 128 + 512 * bank:
                                           128 + 512 * (bank + 1)],
                                 start=True, stop=False)

            ob = [opool.tile([128, 512], f16, name=f"ob{b}", tag=f"ob{b}")
                  for b in range(2)]
            for bank in range(2):
                for qp in range(2):             # col-slot pair index
                    for i in range(10):
                        lo, hi, cofs = _BLK[i]
                        wd = 64 * (hi - lo + 1)
                        for qq in range(2):
                            q = 2 * qp + qq
                            g = 4 * bank + q
                            t = 8 * g + i
                            last = (qp == 1 and i == 9 and qq == 1)
                            nc.tensor.matmul(
                                ps[bank][32 * q + 0:32 * q + 32,
                                         64 * lo:64 * (hi + 1)],
                                lhsT=xr[:, t, :],
                                rhs=w_sb[g][:, 0, cofs:cofs + wd],
                                start=False, stop=last,
                                tile_position=(0, 32 * q),
                            )
                if bank == 0:
                    nc.vector.tensor_copy(out=ob[0][:, :], in_=ps[0][:, :])
                    nc.scalar.dma_start(out=out.ap()[0, :, :],
                                        in_=ob[0][:, :])
                else:
                    # split the final evacuation so its two out-DMAs issue on
                    # different engines concurrently
                    nc.vector.tensor_copy(out=ob[1][:, 0:256],
                                          in_=ps[1][:, 0:256])
                    nc.sync.dma_start(out=out.ap()[1, :, 0:256],
                                      in_=ob[1][:, 0:256])
                    nc.vector.tensor_copy(out=ob[1][:, 256:512],
                                          in_=ps[1][:, 256:512])
                    nc.scalar.dma_start(out=out.ap()[1, :, 256:512],
                                        in_=ob[1][:, 256:512])
    nc.compile()
    return nc


def prep_inputs(input, weight, bias, w_dt=W_DT):
    """Host-side shard + relayout. Returns list of per-core input dicts."""
    wnp = _np_dt(w_dt)
    xpad = np.pad(np.asarray(input, np.float32), ((0, 0), (0, 0), (1, 1)))
    w = np.asarray(weight, np.float32).transpose(1, 2, 3, 0)  # (c, s, k, o)
    bias = np.asarray(bias, np.float32)

    ones = np.zeros((8, 128), np.float16)
    for r in range(8):
        m0 = 32 * (r // 2) + 16 * (r % 2)
        ones[r, m0:m0 + 16] = 1.0

    in_maps = []
    for core in range(N_CORES):
        s0 = core * SC
        xz = np.zeros((128, TW, 2 * B), np.float32)
        xz[0:64, :, 0:B] = xpad[:, :, s0:s0 + TW].transpose(1, 2, 0)
        xz[64:128, :, B:2 * B] = \
            xpad[:, :, s0 + H:s0 + H + TW].transpose(1, 2, 0)

        wq = np.empty((128, NG, GCOLS), np.float32)
        for g in range(NG):
            for i in range(10):
                lo, hi, cofs = _BLK[i]
                for slot in range(lo, hi + 1):
                    j = 8 * g + slot
                    k = i - slot
                    c0 = cofs + (slot - lo) * 64
                    wq[0:64, g, c0:c0 + 64] = w[:, s0 + j, k, :]
                    wq[64:128, g, c0:c0 + 64] = w[:, s0 + H + j, k, :]

        cb = np.empty((8, 128 + 1024), np.float16)
        cb[:, 0:128] = ones
        for r in range(8):
            q, half = r // 2, r % 2
            for bank in range(2):
                sl = s0 + 32 * bank + 8 * q + 64 * half
                cb[r, 128 + 512 * bank:128 + 512 * (bank + 1)] = \
                    bias[:, sl:sl + 8].T.reshape(512)

        in_maps.append({
            "xz": np.ascontiguousarray(xz.astype(wnp)),
            "wq": np.ascontiguousarray(wq.astype(wnp)),
            "cb": cb,
        })
    return in_maps


def assemble_output(results):
    full = np.empty((B, COUT, S), np.float32)
    for core, r in enumerate(results):
        s0 = core * SC
        oc = np.asarray(r["out"], np.float32)     # (2, 128, 512)
        oc = oc.reshape(2, 4, 2, B, 8, COUT)      # bank q half b slot o
        oc = oc.transpose(3, 5, 2, 0, 1, 4)       # b o half bank q slot
        full[:, :, s0:s0 + SC] = oc.reshape(B, COUT, SC)
    return full


_CACHED = {}


def run(inputs, w_dt=W_DT, trace=False):
    if w_dt not in _CACHED:
        _CACHED[w_dt] = build_bass(w_dt)
    nc = _CACHED[w_dt]
    in_maps = prep_inputs(inputs["input"], inputs["weight"], inputs["bias"],
                          w_dt)
    res = bass_utils.run_bass_kernel_spmd(
        nc, in_maps, core_ids=list(range(N_CORES)), trace=trace)
    return assemble_output(res.results), res


def kernel(input, weight, bias):
    out, _ = run({"input": input, "weight": weight, "bias": bias},
                 trace=False)
    return out
